# revision 7
# baseline (speedup 1.0000x reference)
"""Trainium2 Bass kernel for nn_BaselineDNN (ragged embedding-bag + MLP).

v5: startup/teardown trimming on top of v4's fp8 slot-stream design.

Per-core pipeline (8-way data parallel over the batch):
  - Host: fuse weights once: T1 = emb_table @ W1.T  [V, 128] (the masked
    mean commutes with the first linear layer).
  - Host: globally sort batches by length desc, deal round-robin to cores
    so the canonical (max-over-cores) per-batch slot counts are tight
    (<0.1% padding) and all 8 cores share ONE program (SPMD).
  - Host: materialize each core's token rows (T1[x], fp8e4) as a
    contiguous batch-sorted slot stream in DRAM, [128, T*128] with slot
    s <-> (tile s//128, partition s%128).
  - Device: stream row tiles; fp8 staircase matmuls against a host-built
    0/1 mask accumulate per-batch SUMS in f32 PSUM. The 1/len scaling is
    applied exactly in f32 by a DVE multiply in the tail, then
    relu(+b1) -> W2 (bf16) -> sigmoid(+b2) -> per-bank y DMA.

v5 changes (trace-driven):
  - rows group 0 is small (8 tiles) and issued first: first stream
    matmul at ~8us instead of ~12.4us.
  - mask ships in one DMA on the sync (SP) queue; inv ships as [1, Bc]
    (2KB) and is partition-broadcast by the DMA engines - both off the
    rows stream's scalar queue, so no mid-stream bandwidth steal.
  - zero-row for PSUM-zeroing matmuls comes from a DVE memset (no DMA);
    the zeroing matmuls double as PE p-state warmup from ~6.1us.
  - both activation tables (relu, sigmoid) preload via dummy ACTIVATEs
    at startup - the lazy sigmoid table load used to land mid-stream on
    a DMA engine and straggle the rows stream by ~4us.
  - variable bank sizes (256,256,256,224,32): the final bank's tail
    chain (mul/relu/W2/sigmoid) shrinks 8x, and each bank's y slice
    DMAs out right after its sigmoid.
"""

import os
from contextlib import ExitStack

import ml_dtypes
import numpy as np

import concourse.bass as bass
import concourse.bacc as bacc
import concourse.mybir as mybir
import concourse.tile as tile
from concourse._compat import get_trn_type
from concourse.bass_utils import run_bass_kernel_spmd

NCORES = 8
P = 128            # partitions
G0 = 8             # tiles in the first rows group (small: starts PE early)
GTILES = 32        # row tiles per steady-state dma_start (4KB/partition)
# bank column boundaries over the Bc=1024 per-core batches; the last bank
# is small so the final tail chain after the last stream matmul is short
BB = [0, 256, 512, 768, 992, 1024]

LAST_RESULT = None  # BassKernelResults of the most recent run (for test.py)

_NC_CACHE = {}

BF16 = ml_dtypes.bfloat16
FP8 = ml_dtypes.float8_e4m3


def _bank_of(k):
    for b in range(len(BB) - 1):
        if k < BB[b + 1]:
            return b
    raise ValueError(k)


def _build_structure(q):
    """Canonical staircase from per-batch-row slot counts q [Bc].

    Slot stream: batch-row k owns slots S[k]..S[k]+q[k]-1. Tile j =
    slots j*128..j*128+127 spans batch rows kf[j]..kl[j]."""
    Bc = len(q)
    assert Bc == BB[-1]
    S = np.zeros(Bc + 1, np.int64)
    S[1:] = np.cumsum(q)
    total = int(S[-1])
    T = (total + P - 1) // P

    starts = np.arange(T, dtype=np.int64) * P
    ends = np.minimum(starts + P - 1, total - 1)
    kf = np.searchsorted(S, starts, "right") - 1
    kl = np.searchsorted(S, ends, "right") - 1

    w = kl - kf + 1
    moff = np.zeros(T + 1, np.int64)
    moff[1:] = np.cumsum(w)
    Wtot = int(moff[-1])

    nbank = len(BB) - 1
    last_tile = {}
    for j in range(T):
        for b in range(_bank_of(kf[j]), _bank_of(kl[j]) + 1):
            last_tile[b] = j

    parts = []  # per tile: list of (bank, c0, c1, mask_local_off, stop)
    for j in range(T):
        pj = []
        for b in range(_bank_of(kf[j]), _bank_of(kl[j]) + 1):
            kb0 = max(kf[j], BB[b])
            kb1 = min(kl[j], BB[b + 1] - 1)
            pj.append((b, kb0 - BB[b], kb1 - BB[b] + 1,
                       kb0 - kf[j], j == last_tile[b]))
        parts.append(pj)

    return dict(Bc=Bc, S=S, total=total, T=T, kf=kf, kl=kl,
                moff=moff, Wtot=Wtot, nbank=nbank, parts=parts)


def _trace_nc(st, DP):
    """Build + compile the SPMD Bacc program; DP = projected dim (128)."""
    Bc, T, Wtot = st["Bc"], st["T"], st["Wtot"]
    moff, parts, nbank = st["moff"], st["parts"], st["nbank"]
    f32 = mybir.dt.float32
    bf16 = mybir.dt.bfloat16
    fp8 = mybir.dt.float8e4
    assert DP == P

    nc = bacc.Bacc(
        get_trn_type() or "TRN2",
        target_bir_lowering=False,
        debug=False,
        num_devices=NCORES,
    )
    rows_d = nc.dram_tensor("rows", [P, T * P], fp8, kind="ExternalInput")
    mask_d = nc.dram_tensor("mask", [P, Wtot], fp8, kind="ExternalInput")
    inv_d = nc.dram_tensor("invl", [1, Bc], bf16, kind="ExternalInput")
    bias_d = nc.dram_tensor("bias", [P, 2], f32, kind="ExternalInput")
    w2t_d = nc.dram_tensor("w2t", [P, 1], bf16, kind="ExternalInput")
    y_d = nc.dram_tensor("y", [1, Bc], f32, kind="ExternalOutput")

    with tile.TileContext(nc) as tc, ExitStack() as ctx:
        consts = ctx.enter_context(tc.tile_pool(name="consts", bufs=1))
        rpool = ctx.enter_context(tc.tile_pool(name="rows", bufs=8))
        psum = ctx.enter_context(tc.tile_pool(name="psum", bufs=1, space="PSUM"))
        sb = ctx.enter_context(tc.tile_pool(name="sb", bufs=1))

        # Rows group 0 first: SWDGE issue + queue-arm latency is ~1.5us,
        # so this is the long pole for the first stream matmul.
        rt0 = rpool.tile([P, GTILES, P], fp8, tag="rt")
        gl0 = min(G0, T)
        nc.gpsimd.dma_start(out=rt0[:, :gl0, :], in_=rows_d.ap()[:, :gl0 * P])

        # Both activation tables preload via dummy ACTIVATEs, before the
        # scalar queue's DMA issues. Lazily-loaded tables otherwise fetch
        # their DRAM image mid-stream and straggle a DMA engine.
        dum = consts.tile([1, 2], f32)
        nc.vector.memset(dum[:], 0.0)
        nc.scalar.activation(dum[0:1, 0:1], dum[0:1, 0:1],
                             mybir.ActivationFunctionType.Relu)
        nc.scalar.activation(dum[0:1, 1:2], dum[0:1, 1:2],
                             mybir.ActivationFunctionType.Sigmoid)

        # Consts on the scalar HWDGE queue (small, early).
        bias_sb = consts.tile([P, 2], f32)
        nc.scalar.dma_start(out=bias_sb[:], in_=bias_d.ap())
        w2t_sb = consts.tile([P, 1], bf16)
        nc.scalar.dma_start(out=w2t_sb[:], in_=w2t_d.ap())

        # Mask + broadcast inv on the sync (SP) queue: parallel to both
        # the rows stream (gpsimd) and the consts (scalar).
        mask_sb = consts.tile([P, Wtot], fp8)
        nc.sync.dma_start(out=mask_sb[:], in_=mask_d.ap())
        inv_sb = consts.tile([P, Bc], bf16)
        nc.sync.dma_start(out=inv_sb[:],
                          in_=inv_d.ap().to_broadcast((P, Bc)))

        # Zero row from DVE (no DMA): gates only on the memset, so the
        # PSUM-zeroing matmuls run at ~6.1us and warm the PE p-state.
        zrow = consts.tile([1, 512], bf16)
        nc.vector.memset(zrow[:], 0.0)

        # rep_ps[b] accumulates (W1 @ rep_sum).T : [128 h, bank batches].
        # One PSUM tile per logical bank: the sim's accumulation-group
        # tracker requires stops not to interleave within a tile.
        rep_ps = [psum.tile([P, BB[b + 1] - BB[b]], f32,
                            tag=f"rep{b}", name=f"rep{b}")
                  for b in range(nbank)]

        def rep_ap(b):
            return rep_ps[b]

        # Open each PSUM accumulation group with a bank-wide zeroing matmul
        # (K=1, bf16) so every staircase matmul is a pure accumulate.
        for b in range(nbank):
            nc.tensor.matmul(
                rep_ps[b][:], zrow[0:1, 0:P], zrow[0:1, :BB[b + 1] - BB[b]],
                start=True, stop=False,
            )

        # Per-bank tail: h = relu(rep_sum * invlen + b1) in bf16;
        # y = sigmoid(W2 @ h + b2); y slice DMAs out immediately.
        h2 = sb.tile([P, Bc], bf16)
        l_ps = psum.tile([1, Bc], f32, tag="lps", name="lps")
        y_sb = sb.tile([1, Bc], f32)
        close_at = {}
        for j in range(T):
            for (b, _c0, _c1, _ml, sp_flag) in parts[j]:
                if sp_flag:
                    close_at.setdefault(j, []).append(b)

        # Stage 1 of a bank's tail (DVE mult + Scalar relu) is emitted at
        # the closing tile; stage 2 (PE W2 matmul + sigmoid + y DMA) is
        # deferred two DMA groups so the relu is certainly finished before
        # the PE's queue reaches the W2 matmul.
        hms = {}

        def emit_tail1(b):
            cols = slice(BB[b], BB[b + 1])
            hm = sb.tile([P, BB[b + 1] - BB[b]], f32, tag=f"hm{b}",
                         name=f"hm{b}")
            hms[b] = hm
            nc.vector.tensor_mul(hm[:], rep_ap(b)[:], inv_sb[:, cols])

        def emit_tail2(b):
            cols = slice(BB[b], BB[b + 1])
            hm = hms[b]
            nc.scalar.activation(
                h2[:, cols], hm[:],
                mybir.ActivationFunctionType.Relu,
                bias=bias_sb[:, 0:1],
            )
            nc.tensor.matmul(
                l_ps[:, cols], w2t_sb[:], h2[:, cols],
                start=True, stop=True,
            )
            nc.scalar.activation(
                y_sb[:, cols], l_ps[:, cols],
                mybir.ActivationFunctionType.Sigmoid,
                bias=bias_sb[0:1, 1:2],
            )
            nc.scalar.dma_start(out=y_d.ap()[:, cols], in_=y_sb[:, cols])

        # group boundaries: [0, G0), then GTILES-sized
        gb = [0, min(G0, T)]
        while gb[-1] < T:
            gb.append(min(gb[-1] + GTILES, T))

        pending2 = []  # (group index when stage-2 may be emitted, bank)
        for gi in range(len(gb) - 1):
            t0, t1 = gb[gi], gb[gi + 1]
            gl = t1 - t0
            if gi == 0:
                rt = rt0
            else:
                rt = rpool.tile([P, GTILES, P], fp8, tag="rt")
                nc.gpsimd.dma_start(
                    out=rt[:, :gl, :],
                    in_=rows_d.ap()[:, t0 * P:t1 * P],
                )
            while pending2 and pending2[0][0] <= gi:
                emit_tail2(pending2.pop(0)[1])
            for jl in range(gl):
                j = t0 + jl
                mo = int(moff[j])
                lhsT = rt[:, jl, :]
                for (b, c0, c1, ml, sp_flag) in parts[j]:
                    nc.tensor.matmul(
                        rep_ap(b)[:, c0:c1],
                        lhsT,
                        mask_sb[:, mo + ml: mo + ml + (c1 - c0)],
                        start=False,
                        stop=sp_flag,
                    )
                for b in close_at.get(j, ()):
                    emit_tail1(b)
                    pending2.append((gi + 2, b))
        for _g, b in pending2:
            emit_tail2(b)

    nc.compile()
    return nc


def _prepare(x, lengths, emb_table, W1, b1, W2, b2):
    """Host-side sharding: weight fusion + canonical structure + arrays."""
    x = np.asarray(x)
    lengths = np.asarray(lengths).astype(np.int64)
    B, L = x.shape
    V, D = emb_table.shape
    Bc = B // NCORES

    # weight fusion: masked-mean commutes with W1
    W1f = np.asarray(W1, np.float32)
    t1 = np.ascontiguousarray(
        np.asarray(emb_table, np.float32) @ W1f.T)     # [V, 128]
    DP = t1.shape[1]
    t1q = t1.astype(FP8)

    # Sort by length desc, deal round-robin: row k of perm holds 8 batches
    # of near-equal length, so the canonical per-row slot count
    # q[k] = max_c len is tight.
    order = np.argsort(-lengths, kind="stable")
    perm = order.reshape(Bc, NCORES)          # [k, core] -> original batch idx
    plen = lengths[perm]                      # [k, core]
    q = plen.max(axis=1)                      # [Bc]

    st = _build_structure(q)
    S, T = st["S"], st["T"]
    kf, moff, Wtot = st["kf"], st["moff"], st["Wtot"]
    TS = T * P

    lpos = np.arange(L, dtype=np.int64)
    kk_base = np.arange(Bc, dtype=np.int64)

    in_maps = []
    bias = np.zeros((P, 2), np.float32)
    bias[:, 0] = np.asarray(b1, np.float32)
    bias[0, 1] = float(np.asarray(b2, np.float32).reshape(-1)[0])
    w2t = np.ascontiguousarray(
        np.asarray(W2, np.float32).reshape(1, P).T).astype(BF16)

    for core in range(NCORES):
        lc = plen[:, core]
        xc = x[perm[:, core]]
        validc = lpos[None, :] < lc[:, None]
        tok = xc[validc]                      # valid ids in (k, l) order
        nv = int(lc.sum())
        kk = np.repeat(kk_base, lc)
        csl = np.zeros(Bc + 1, np.int64)
        csl[1:] = np.cumsum(lc)
        ofs = np.arange(nv, dtype=np.int64) - np.repeat(csl[:-1], lc)
        slot = S[kk] + ofs

        # rows: slot s -> (tile s//128, partition s%128); DRAM layout
        # [128, T*128] with partition p holding its slots contiguously.
        rows_all = np.zeros((TS, DP), FP8)
        rows_all[slot] = t1q[tok]
        rows = np.ascontiguousarray(
            rows_all.reshape(T, P, DP).transpose(1, 0, 2).reshape(P, T * DP))

        # mask: exact 1.0 at (slot%128, staircase column of (tile, k))
        tile_s = slot // P
        col = moff[tile_s] + (kk - kf[tile_s])
        mask_host = np.zeros((P, Wtot), FP8)
        mask_host[slot % P, col] = FP8(1.0)

        inv = (1.0 / lc.astype(np.float64)).astype(np.float32).astype(BF16)

        in_maps.append({
            "rows": rows,
            "mask": mask_host,
            "invl": np.ascontiguousarray(inv.reshape(1, Bc)),
            "bias": bias,
            "w2t": w2t,
        })
    return st, perm, in_maps, DP


def kernel(x, lengths, emb_table, W1, b1, W2, b2):
    global LAST_RESULT
    st, perm, in_maps, DP = _prepare(x, lengths, emb_table, W1, b1, W2, b2)

    key = (st["T"], st["Wtot"], st["Bc"], DP,
           hash(st["kf"].tobytes()), hash(st["kl"].tobytes()))
    nc = _NC_CACHE.get(key)
    if nc is None:
        nc = _trace_nc(st, DP)
        _NC_CACHE[key] = nc

    trace = bool(int(os.environ.get("KERNEL_TRACE", "0")))
    res = run_bass_kernel_spmd(nc, in_maps, core_ids=list(range(NCORES)),
                               trace=trace)
    LAST_RESULT = res

    B = perm.size
    out = np.zeros(B, np.float32)
    for c in range(NCORES):
        out[perm[:, c]] = res.results[c]["y"][0]
    return out
